# revision 20
# baseline (speedup 1.0000x reference)
"""OLMoE sparse-MoE block on 8 TRN2 NeuronCores, expert-parallel.

Strategy
--------
E=32 experts are sharded 4-per-core across 8 cores. The router
(T=1024 tokens x E logits -> softmax -> top-4) is computed on the host
in fp32 numpy as part of input sharding: the token->expert dispatch
("all-to-all") is done by gathering each expert's tokens on the host
and shipping each core only the tokens its experts need. Experts are
rank-banded by token count so that each slot index has a tight
capacity (SPMD needs identical shapes per core, but capacities may
differ across the 4 slots).

Each core runs a Bass/Tile kernel computing, per expert slot s:
    g^T = Wg_s^T x_s^T, u^T = Wu_s^T x_s^T      (activations kept
    t^T = silu(g^T) * u^T                        feature-major, tokens
    y   = (t^T)^T Wd_s  scaled by routing weight on the free axis)
All matmuls run in bf16 with fp32 PSUM accumulation; the scaled y is
written back in bf16 and scatter-added on the host in fp32.

The kernel sits at the memory/compute roofline ridge: ~51 MB of
HBM traffic per core against ~150 us of TensorEngine time. Loads are
issued in consumption order on the sync engine (weights) and gpsimd
(tokens); stores go through the scalar engine so they never block the
load stream.
"""

import sys

if "/opt/trn_rl_repo" not in sys.path:
    sys.path.insert(0, "/opt/trn_rl_repo")

import ml_dtypes
import numpy as np

import concourse.bass as bass
import concourse.tile as tile
from concourse import bacc, mybir
from concourse import bass_utils

# ---------------------------------------------------------------- config
E, K = 32, 4
H, F = 2048, 1024
N_CORES = 8
NSLOT = E // N_CORES  # expert slots per core
HC = H // 128  # 16 h-chunks
FC = F // 128  # 8 f-chunks

BF16 = mybir.dt.bfloat16
F32 = mybir.dt.float32
NP_BF16 = ml_dtypes.bfloat16

# set by test.py to capture a profile; ignored by normal calls
TRACE = False
LAST_RESULT = None

_kernel_cache = {}


# ---------------------------------------------------------------- device
def _build(caps):
    """Per-core Tile kernel; caps[s] = token capacity of expert slot s."""
    assert all(c % 8 == 0 and c <= 256 for c in caps)
    ncgs = [(c + 127) // 128 for c in caps]
    rw_cols = sum(ncgs)

    nc = bacc.Bacc("TRN2", target_bir_lowering=False, debug=False,
                   num_devices=N_CORES)
    xTs = [
        nc.dram_tensor(f"xT{s}", [HC, 128, caps[s]], BF16,
                       kind="ExternalInput").ap()
        for s in range(NSLOT)
    ]
    wg = nc.dram_tensor("wg", [NSLOT, H, F], BF16, kind="ExternalInput").ap()
    wu = nc.dram_tensor("wu", [NSLOT, H, F], BF16, kind="ExternalInput").ap()
    wd = nc.dram_tensor("wd", [NSLOT, F, H], BF16, kind="ExternalInput").ap()
    rw = nc.dram_tensor("rw", [128, rw_cols], F32, kind="ExternalInput").ap()
    ys = [
        nc.dram_tensor(f"y{s}", [caps[s], H], BF16, kind="ExternalOutput").ap()
        for s in range(NSLOT)
    ]

    with tile.TileContext(nc) as tc:
        with (
            tc.tile_pool(name="xp", bufs=8) as xp,
            tc.tile_pool(name="wp", bufs=16) as wp,
            tc.tile_pool(name="wdp", bufs=20) as wdp,
            tc.tile_pool(name="tp", bufs=2 * FC) as tp,
            tc.tile_pool(name="sp", bufs=3) as sp,
            tc.tile_pool(name="yp", bufs=6) as yp,
            tc.tile_pool(name="rwp", bufs=1) as rwp,
            tc.tile_pool(name="ps", bufs=8, space="PSUM") as ps,
        ):
            rwsb = rwp.tile([128, rw_cols], F32, tag="rw", name="rwsb")
            nc.sync.dma_start(rwsb[:], rw[:])
            for s in range(NSLOT):
                C = caps[s]
                n_cg = ncgs[s]
                rw_off = sum(ncgs[:s])

                # ---- token tiles (feature-major), 4 h-chunks per descriptor
                xt4s = []
                for a in range(HC // 4):
                    xt4 = xp.tile([128, 4, C], BF16, tag="xt", name=f"xt{s}_{a}")
                    nc.gpsimd.dma_start(
                        xt4[:], xTs[s][4 * a : 4 * a + 4].rearrange("a p c -> p a c")
                    )
                    xt4s.append(xt4)
                xts = [xt4s[h // 4][:, h % 4, :] for h in range(HC)]

                # ---- phase 1: g/u accumulation over h-chunks
                gu = [ps.tile([128, 2 * C], F32, tag="ps", name=f"gu{s}_{f}")
                      for f in range(FC)]
                for h in range(HC):
                    wgt = wp.tile([128, F], BF16, tag="w", name=f"wg{s}_{h}")
                    wut = wp.tile([128, F], BF16, tag="w", name=f"wu{s}_{h}")
                    if s == 0 and h == 0:
                        # fan the pipeline-head tiles over 4 queues
                        # (row-contiguous 64KB chunks) so the first matmul
                        # starts sooner
                        for q in range(4):
                            rs = slice(q * 32, (q + 1) * 32)
                            nc.sync.dma_start(
                                wgt[rs, :],
                                wg[s, h * 128 + q * 32 : h * 128 + (q + 1) * 32, :],
                            )
                            nc.sync.dma_start(
                                wut[rs, :],
                                wu[s, h * 128 + q * 32 : h * 128 + (q + 1) * 32, :],
                            )
                    else:
                        nc.sync.dma_start(wgt[:], wg[s, h * 128 : (h + 1) * 128, :])
                        nc.sync.dma_start(wut[:], wu[s, h * 128 : (h + 1) * 128, :])
                    # one accumulation group per PSUM bank (start=True zeroes
                    # the whole 2KB zero-region); all g then all u so
                    # consecutive matmuls never hit the same bank.
                    for f in range(FC):
                        nc.tensor.matmul(
                            gu[f][:, 0:C], wgt[:, f * 128 : (f + 1) * 128],
                            xts[h], start=(h == 0), stop=False,
                        )
                    for f in range(FC):
                        nc.tensor.matmul(
                            gu[f][:, C : 2 * C], wut[:, f * 128 : (f + 1) * 128],
                            xts[h], start=False, stop=(h == HC - 1),
                        )

                # ---- t = silu(g) * u  (bf16, feature-major)
                tts = []
                for f in range(FC):
                    sg = sp.tile([128, C], F32, tag="sg", name=f"sg{s}_{f}")
                    nc.scalar.activation(
                        sg[:], gu[f][:, 0:C], mybir.ActivationFunctionType.Silu
                    )
                    tt = tp.tile([128, C], BF16, tag="tt", name=f"tt{s}_{f}")
                    nc.vector.tensor_mul(tt[:], sg[:], gu[f][:, C : 2 * C])
                    tts.append(tt)

                # ---- phase 2: y = t @ Wd, scaled by routing weight
                wdts = []
                for f in range(FC):
                    wdt = wdp.tile([128, H], BF16, tag="wd", name=f"wd{s}_{f}")
                    nc.sync.dma_start(wdt[:, 0 : H // 2], wd[s, f * 128 : (f + 1) * 128, 0 : H // 2])
                    nc.sync.dma_start(wdt[:, H // 2 : H], wd[s, f * 128 : (f + 1) * 128, H // 2 : H])
                    wdts.append(wdt)
                for cg in range(n_cg):
                    cgs = min(128, C - cg * 128)
                    # interleave the 4 output banks so consecutive matmuls
                    # never accumulate into the same PSUM bank (fill overlaps
                    # drain)
                    pys = [ps.tile([128, 512], F32, tag="ps", name=f"py{s}_{cg}_{hg}")
                           for hg in range(H // 512)]
                    for f in range(FC):
                        for hg in range(H // 512):
                            nc.tensor.matmul(
                                pys[hg][:cgs, :],
                                tts[f][:, cg * 128 : cg * 128 + cgs],
                                wdts[f][:, hg * 512 : (hg + 1) * 512],
                                start=(f == 0), stop=(f == FC - 1),
                            )
                    for hg in range(H // 512):
                        ysb = yp.tile([128, 512], BF16, tag="y", name=f"y{s}_{cg}_{hg}")
                        nc.vector.tensor_scalar_mul(
                            ysb[:cgs, :], pys[hg][:cgs, :],
                            rwsb[:cgs, rw_off + cg : rw_off + cg + 1],
                        )
                        # store from the scalar engine: compute-gated stores
                        # must never block the sync engine's load stream
                        nc.scalar.dma_start(
                            ys[s][cg * 128 : cg * 128 + cgs,
                                  hg * 512 : (hg + 1) * 512],
                            ysb[:cgs, :],
                        )

    nc.compile()
    return nc


def _get_kernel(caps):
    caps = tuple(caps)
    if caps not in _kernel_cache:
        _kernel_cache[caps] = _build(caps)
    return _kernel_cache[caps]


# ------------------------------------------------------------------ host
def _softmax(x):
    m = x.max(axis=-1, keepdims=True)
    p = np.exp(x - m)
    return p / p.sum(axis=-1, keepdims=True)


def kernel(hidden_states, gate_w, w_gate, w_up, w_down):
    b, s_len, h = hidden_states.shape
    T = b * s_len
    x = np.asarray(hidden_states, np.float32).reshape(T, h)
    gate_w = np.asarray(gate_w, np.float32)

    # router (fp32, host): logits -> softmax -> top-4
    router_logits = x @ gate_w.T  # [T, E]
    probs = _softmax(router_logits)
    topk_idx = np.argsort(-probs, axis=-1, kind="stable")[:, :K]  # [T, K]
    topk_val = np.take_along_axis(probs, topk_idx, axis=-1)  # [T, K]

    # token lists per expert
    tok = [[] for _ in range(E)]
    wgt = [[] for _ in range(E)]
    for k in range(K):
        for t in range(T):
            e = topk_idx[t, k]
            tok[e].append(t)
            wgt[e].append(topk_val[t, k])
    counts = np.array([len(t) for t in tok])

    # rank-band experts by count: slot s takes ranks [s*8, s*8+8) so each
    # slot's capacity hugs the max count in its band
    order = np.argsort(-counts, kind="stable")
    slot_expert = [[0] * NSLOT for _ in range(N_CORES)]
    caps = []
    for s in range(NSLOT):
        band = order[s * N_CORES : (s + 1) * N_CORES]
        for c in range(N_CORES):
            slot_expert[c][s] = int(band[c])
        caps.append(int(min(256, max(32, ((counts[band].max() + 7) // 8) * 8))))
    ncgs = [(c + 127) // 128 for c in caps]
    rw_cols = sum(ncgs)

    # tokens beyond a slot's capacity (only possible when a count exceeds
    # 256) are handled by extra passes
    n_pass = max(1, int(np.ceil(counts.max() / min(caps))))

    x_bf = x.astype(NP_BF16)
    wg_bf = np.asarray(w_gate).astype(NP_BF16)
    wu_bf = np.asarray(w_up).astype(NP_BF16)
    wd_bf = np.asarray(w_down).astype(NP_BF16)

    nc = _get_kernel(caps)
    out = np.zeros((T, h), np.float32)

    global LAST_RESULT
    for p in range(n_pass):
        in_maps = []
        meta = []  # per core: list of (expert, tokens)
        for c in range(N_CORES):
            rw_arr = np.zeros((128, rw_cols), np.float32)
            wg_arr = np.empty((NSLOT, H, F), NP_BF16)
            wu_arr = np.empty((NSLOT, H, F), NP_BF16)
            wd_arr = np.empty((NSLOT, F, H), NP_BF16)
            im = {"wg": wg_arr, "wu": wu_arr, "wd": wd_arr}
            cmeta = []
            for s in range(NSLOT):
                C = caps[s]
                e = slot_expert[c][s]
                tks = tok[e][p * C : (p + 1) * C]
                wts = wgt[e][p * C : (p + 1) * C]
                n = len(tks)
                xT_arr = np.zeros((HC, 128, C), NP_BF16)
                if n:
                    xe = x_bf[tks].T  # [H, n]
                    xT_arr[:, :, :n] = xe.reshape(HC, 128, n)
                    wcol = np.zeros(C, np.float32)
                    wcol[:n] = wts
                    off = sum(ncgs[:s])
                    for cg in range(ncgs[s]):
                        cgs = min(128, C - cg * 128)
                        rw_arr[:cgs, off + cg] = wcol[cg * 128 : cg * 128 + cgs]
                im[f"xT{s}"] = xT_arr
                wg_arr[s] = wg_bf[e]
                wu_arr[s] = wu_bf[e]
                wd_arr[s] = wd_bf[e]
                cmeta.append((e, tks))
            im["rw"] = rw_arr
            in_maps.append(im)
            meta.append(cmeta)

        res = bass_utils.run_bass_kernel_spmd(
            nc, in_maps, list(range(N_CORES)), trace=TRACE
        )
        LAST_RESULT = res

        for c in range(N_CORES):
            for s in range(NSLOT):
                e, tks = meta[c][s]
                n = len(tks)
                if n:
                    y_c = res.results[c][f"y{s}"]  # [C, H], weight-scaled
                    out[tks] += np.asarray(y_c[:n], dtype=np.float32)

    return out.reshape(b, s_len, h), router_logits


# revision 21
# speedup vs baseline: 1.0925x; 1.0925x over previous
"""OLMoE sparse-MoE block on 8 TRN2 NeuronCores, expert-parallel.

Strategy
--------
E=32 experts are sharded 4-per-core across 8 cores. The router
(T=1024 tokens x E logits -> softmax -> top-4) is computed on the host
in fp32 numpy as part of input sharding: the token->expert dispatch
("all-to-all") is done by gathering each expert's tokens on the host
and shipping each core only the tokens its experts need. Experts are
rank-banded by token count so that each slot index has a tight
capacity (SPMD needs identical shapes per core, but capacities may
differ across the 4 slots).

Each core runs a Bass/Tile kernel computing, per expert slot s:
    g^T = Wg_s^T x_s^T, u^T = Wu_s^T x_s^T      (activations kept
    t^T = silu(g^T) * u^T                        feature-major, tokens
    y   = (t^T)^T Wd_s  scaled by routing weight on the free axis)
All matmuls run in bf16 with fp32 PSUM accumulation; the scaled y is
written back in bf16 and scatter-added on the host in fp32.

The kernel sits at the memory/compute roofline ridge: ~51 MB of
HBM traffic per core against ~150 us of TensorEngine time. Loads are
issued in consumption order on the sync engine (weights) and gpsimd
(tokens); stores go through the scalar engine so they never block the
load stream.
"""

import sys

if "/opt/trn_rl_repo" not in sys.path:
    sys.path.insert(0, "/opt/trn_rl_repo")

import ml_dtypes
import numpy as np

import concourse.bass as bass
import concourse.tile as tile
from concourse import bacc, mybir
from concourse import bass_utils

# ---------------------------------------------------------------- config
E, K = 32, 4
H, F = 2048, 1024
N_CORES = 8
NSLOT = E // N_CORES  # expert slots per core
HC = H // 128  # 16 h-chunks
FC = F // 128  # 8 f-chunks

BF16 = mybir.dt.bfloat16
F32 = mybir.dt.float32
NP_BF16 = ml_dtypes.bfloat16

# set by test.py to capture a profile; ignored by normal calls
TRACE = False
LAST_RESULT = None

_kernel_cache = {}


# ---------------------------------------------------------------- device
def _build(caps):
    """Per-core Tile kernel; caps[s] = token capacity of expert slot s."""
    assert all(c % 8 == 0 and c <= 256 for c in caps)
    ncgs = [(c + 127) // 128 for c in caps]
    rw_cols = sum(ncgs)

    nc = bacc.Bacc("TRN2", target_bir_lowering=False, debug=False,
                   num_devices=N_CORES)
    xTs = [
        nc.dram_tensor(f"xT{s}", [HC, 128, caps[s]], BF16,
                       kind="ExternalInput").ap()
        for s in range(NSLOT)
    ]
    wg = nc.dram_tensor("wg", [NSLOT, H, F], BF16, kind="ExternalInput").ap()
    wu = nc.dram_tensor("wu", [NSLOT, H, F], BF16, kind="ExternalInput").ap()
    wd = nc.dram_tensor("wd", [NSLOT, F, H], BF16, kind="ExternalInput").ap()
    rw = nc.dram_tensor("rw", [128, rw_cols], F32, kind="ExternalInput").ap()
    ys = [
        nc.dram_tensor(f"y{s}", [caps[s], H], BF16, kind="ExternalOutput").ap()
        for s in range(NSLOT)
    ]

    with tile.TileContext(nc) as tc:
        with (
            tc.tile_pool(name="xp", bufs=8) as xp,
            tc.tile_pool(name="wp", bufs=12) as wp,
            tc.tile_pool(name="wdp", bufs=16) as wdp,
            tc.tile_pool(name="tp", bufs=2 * FC) as tp,
            tc.tile_pool(name="sp", bufs=3) as sp,
            tc.tile_pool(name="yp", bufs=6) as yp,
            tc.tile_pool(name="rwp", bufs=1) as rwp,
            tc.tile_pool(name="ps", bufs=8, space="PSUM") as ps,
        ):
            rwsb = rwp.tile([128, rw_cols], F32, tag="rw", name="rwsb")
            nc.sync.dma_start(rwsb[:], rw[:])
            for s in range(NSLOT):
                C = caps[s]
                n_cg = ncgs[s]
                rw_off = sum(ncgs[:s])

                # ---- token tiles (feature-major), 4 h-chunks per descriptor
                xt4s = []
                for a in range(HC // 4):
                    xt4 = xp.tile([128, 4, C], BF16, tag="xt", name=f"xt{s}_{a}")
                    nc.gpsimd.dma_start(
                        xt4[:], xTs[s][4 * a : 4 * a + 4].rearrange("a p c -> p a c")
                    )
                    xt4s.append(xt4)
                xts = [xt4s[h // 4][:, h % 4, :] for h in range(HC)]

                # ---- phase 1: g/u accumulation over h-chunks
                gu = [ps.tile([128, 2 * C], F32, tag="ps", name=f"gu{s}_{f}")
                      for f in range(FC)]
                for h in range(HC):
                    wgt = wp.tile([128, F], BF16, tag="w", name=f"wg{s}_{h}")
                    wut = wp.tile([128, F], BF16, tag="w", name=f"wu{s}_{h}")
                    if s == 0 and h == 0:
                        # fan the pipeline-head tiles over 4 queues
                        # (row-contiguous 64KB chunks) so the first matmul
                        # starts sooner
                        for q in range(4):
                            rs = slice(q * 32, (q + 1) * 32)
                            nc.sync.dma_start(
                                wgt[rs, :],
                                wg[s, h * 128 + q * 32 : h * 128 + (q + 1) * 32, :],
                            )
                            nc.sync.dma_start(
                                wut[rs, :],
                                wu[s, h * 128 + q * 32 : h * 128 + (q + 1) * 32, :],
                            )
                    else:
                        nc.sync.dma_start(wgt[:], wg[s, h * 128 : (h + 1) * 128, :])
                        nc.sync.dma_start(wut[:], wu[s, h * 128 : (h + 1) * 128, :])
                    # one accumulation group per PSUM bank (start=True zeroes
                    # the whole 2KB zero-region); all g then all u so
                    # consecutive matmuls never hit the same bank.
                    for f in range(FC):
                        nc.tensor.matmul(
                            gu[f][:, 0:C], wgt[:, f * 128 : (f + 1) * 128],
                            xts[h], start=(h == 0), stop=False,
                        )
                    for f in range(FC):
                        nc.tensor.matmul(
                            gu[f][:, C : 2 * C], wut[:, f * 128 : (f + 1) * 128],
                            xts[h], start=False, stop=(h == HC - 1),
                        )

                # ---- t = silu(g) * u  (bf16, feature-major)
                tts = []
                for f in range(FC):
                    sg = sp.tile([128, C], F32, tag="sg", name=f"sg{s}_{f}")
                    nc.scalar.activation(
                        sg[:], gu[f][:, 0:C], mybir.ActivationFunctionType.Silu
                    )
                    tt = tp.tile([128, C], BF16, tag="tt", name=f"tt{s}_{f}")
                    nc.vector.tensor_mul(tt[:], sg[:], gu[f][:, C : 2 * C])
                    tts.append(tt)

                # ---- phase 2: y = t @ Wd, scaled by routing weight
                wdts = []
                for f in range(FC):
                    wdt = wdp.tile([128, H], BF16, tag="wd", name=f"wd{s}_{f}")
                    nc.sync.dma_start(wdt[:], wd[s, f * 128 : (f + 1) * 128, :])
                    wdts.append(wdt)
                for cg in range(n_cg):
                    cgs = min(128, C - cg * 128)
                    # interleave the 4 output banks so consecutive matmuls
                    # never accumulate into the same PSUM bank (fill overlaps
                    # drain)
                    pys = [ps.tile([128, 512], F32, tag="ps", name=f"py{s}_{cg}_{hg}")
                           for hg in range(H // 512)]
                    for f in range(FC):
                        for hg in range(H // 512):
                            nc.tensor.matmul(
                                pys[hg][:cgs, :],
                                tts[f][:, cg * 128 : cg * 128 + cgs],
                                wdts[f][:, hg * 512 : (hg + 1) * 512],
                                start=(f == 0), stop=(f == FC - 1),
                            )
                    for hg in range(H // 512):
                        ysb = yp.tile([128, 512], BF16, tag="y", name=f"y{s}_{cg}_{hg}")
                        nc.vector.tensor_scalar_mul(
                            ysb[:cgs, :], pys[hg][:cgs, :],
                            rwsb[:cgs, rw_off + cg : rw_off + cg + 1],
                        )
                        # store from the scalar engine: compute-gated stores
                        # must never block the sync engine's load stream
                        nc.scalar.dma_start(
                            ys[s][cg * 128 : cg * 128 + cgs,
                                  hg * 512 : (hg + 1) * 512],
                            ysb[:cgs, :],
                        )

    nc.compile()
    return nc


def _get_kernel(caps):
    caps = tuple(caps)
    if caps not in _kernel_cache:
        _kernel_cache[caps] = _build(caps)
    return _kernel_cache[caps]


# ------------------------------------------------------------------ host
def _softmax(x):
    m = x.max(axis=-1, keepdims=True)
    p = np.exp(x - m)
    return p / p.sum(axis=-1, keepdims=True)


def kernel(hidden_states, gate_w, w_gate, w_up, w_down):
    b, s_len, h = hidden_states.shape
    T = b * s_len
    x = np.asarray(hidden_states, np.float32).reshape(T, h)
    gate_w = np.asarray(gate_w, np.float32)

    # router (fp32, host): logits -> softmax -> top-4
    router_logits = x @ gate_w.T  # [T, E]
    probs = _softmax(router_logits)
    topk_idx = np.argsort(-probs, axis=-1, kind="stable")[:, :K]  # [T, K]
    topk_val = np.take_along_axis(probs, topk_idx, axis=-1)  # [T, K]

    # token lists per expert
    tok = [[] for _ in range(E)]
    wgt = [[] for _ in range(E)]
    for k in range(K):
        for t in range(T):
            e = topk_idx[t, k]
            tok[e].append(t)
            wgt[e].append(topk_val[t, k])
    counts = np.array([len(t) for t in tok])

    # rank-band experts by count: slot s takes ranks [s*8, s*8+8) so each
    # slot's capacity hugs the max count in its band
    order = np.argsort(-counts, kind="stable")
    slot_expert = [[0] * NSLOT for _ in range(N_CORES)]
    caps = []
    for s in range(NSLOT):
        band = order[s * N_CORES : (s + 1) * N_CORES]
        for c in range(N_CORES):
            slot_expert[c][s] = int(band[c])
        caps.append(int(min(256, max(32, ((counts[band].max() + 7) // 8) * 8))))
    ncgs = [(c + 127) // 128 for c in caps]
    rw_cols = sum(ncgs)

    # tokens beyond a slot's capacity (only possible when a count exceeds
    # 256) are handled by extra passes
    n_pass = max(1, int(np.ceil(counts.max() / min(caps))))

    x_bf = x.astype(NP_BF16)
    wg_bf = np.asarray(w_gate).astype(NP_BF16)
    wu_bf = np.asarray(w_up).astype(NP_BF16)
    wd_bf = np.asarray(w_down).astype(NP_BF16)

    nc = _get_kernel(caps)
    out = np.zeros((T, h), np.float32)

    global LAST_RESULT
    for p in range(n_pass):
        in_maps = []
        meta = []  # per core: list of (expert, tokens)
        for c in range(N_CORES):
            rw_arr = np.zeros((128, rw_cols), np.float32)
            wg_arr = np.empty((NSLOT, H, F), NP_BF16)
            wu_arr = np.empty((NSLOT, H, F), NP_BF16)
            wd_arr = np.empty((NSLOT, F, H), NP_BF16)
            im = {"wg": wg_arr, "wu": wu_arr, "wd": wd_arr}
            cmeta = []
            for s in range(NSLOT):
                C = caps[s]
                e = slot_expert[c][s]
                tks = tok[e][p * C : (p + 1) * C]
                wts = wgt[e][p * C : (p + 1) * C]
                n = len(tks)
                xT_arr = np.zeros((HC, 128, C), NP_BF16)
                if n:
                    xe = x_bf[tks].T  # [H, n]
                    xT_arr[:, :, :n] = xe.reshape(HC, 128, n)
                    wcol = np.zeros(C, np.float32)
                    wcol[:n] = wts
                    off = sum(ncgs[:s])
                    for cg in range(ncgs[s]):
                        cgs = min(128, C - cg * 128)
                        rw_arr[:cgs, off + cg] = wcol[cg * 128 : cg * 128 + cgs]
                im[f"xT{s}"] = xT_arr
                wg_arr[s] = wg_bf[e]
                wu_arr[s] = wu_bf[e]
                wd_arr[s] = wd_bf[e]
                cmeta.append((e, tks))
            im["rw"] = rw_arr
            in_maps.append(im)
            meta.append(cmeta)

        res = bass_utils.run_bass_kernel_spmd(
            nc, in_maps, list(range(N_CORES)), trace=TRACE
        )
        LAST_RESULT = res

        for c in range(N_CORES):
            for s in range(NSLOT):
                e, tks = meta[c][s]
                n = len(tks)
                if n:
                    y_c = res.results[c][f"y{s}"]  # [C, H], weight-scaled
                    out[tks] += np.asarray(y_c[:n], dtype=np.float32)

    return out.reshape(b, s_len, h), router_logits
